# revision 28
# baseline (speedup 1.0000x reference)
"""Trainium2 Bass kernel for a pairwise-distance cluster margin loss.

Key observation: the loss only ever reads SAME-CLASS distances (the mask
selects targets_i == targets_j for both the farthest-positive and the
second-nearest-positive), so the full 4096x4096 distance matrix is
unnecessary. After grouping points by class on the host, only the 64
class-diagonal Gram blocks (~82x82 max, padded to 96) are needed:
~64x less matmul work than the full GEMM.

Per class c (padded to S=96 points, zero-padded cols/rows):
    G    = x_c @ x_c.T                       (fp8 DoubleRow chain, PSUM f32)
    A    = G - sq_v/2 (bf16 hi/lo aug rows; pad cols get +2^20)
    fmin = rowmin(A)      -> far2_u  = sq_u - 2*fmin   (pad cols excluded
                             by +2^20; diag is +sq_u/2, never the min)
    M    = A + bmat        (bmat: diag -2^20, pad cols -2^21, via eye matmul)
    gmax = rowmax(M)      -> near2_u = sq_u - 2*gmax   (diag+pad pushed out)
Host: far=sqrt(max(far2,1e-12)), near=sqrt(max(near2,1e-12)),
loss = mean(relu(far-near)).  8 classes per core, 8 cores.
"""

import numpy as np
import ml_dtypes

BF = ml_dtypes.bfloat16
F8 = ml_dtypes.float8_e4m3

N = 4096
D = 2048
P = 128
NCORES = 8
NCLS = 64
S = 96  # padded class size (max observed 82)
CPC = NCLS // NCORES  # 8 classes per core
KX = D // P  # 16 k-chunks of 128
HUGE = float(2.0**20)

_compiled = None


def _build_nc():
    import concourse.mybir as mybir
    import concourse.tile as tile
    from concourse import bacc

    nc = bacc.Bacc("TRN2", target_bir_lowering=False)
    f32 = mybir.dt.float32
    bf16 = mybir.dt.bfloat16
    fp8 = mybir.dt.float8e4
    DR = mybir.MatmulPerfMode.DoubleRow
    X = mybir.AxisListType.X

    xt8_d = nc.dram_tensor("xt8", [P, CPC, KX, S], fp8, kind="ExternalInput")
    # row 0: ones lhsT rows; rows 1..CPC: per-class aug rhs rows
    ofa_d = nc.dram_tensor("ofa", [2, 1 + CPC, S], bf16, kind="ExternalInput")
    bmat_d = nc.dram_tensor("bmat", [S, CPC, S], bf16, kind="ExternalInput")
    res_d = nc.dram_tensor("res", [S, 2, CPC], f32, kind="ExternalOutput")

    with tile.TileContext(nc) as tc:
        with (
            tc.tile_pool(name="singles", bufs=1) as singles,
            tc.tile_pool(name="ps", bufs=1, space="PSUM") as psp,
        ):
            # two HWDGE queues, ordered by compute need-time: aug rows and
            # slot0 first, then slots interleaved, bmat before first tadd
            ofa = singles.tile([2, 1 + CPC, S], bf16)
            nc.sync.dma_start(out=ofa, in_=ofa_d[:, :, :])
            ones2 = ofa[:, 0, :]
            faug = ofa[:, 1:, :]
            xt8 = singles.tile([P, CPC, KX, S], fp8)
            nc.scalar.dma_start(
                out=xt8[:, 0:1, 0:8, :], in_=xt8_d[:, 0:1, 0:8, :]
            )
            nc.scalar.dma_start(
                out=xt8[:, 0:1, 8:KX, :], in_=xt8_d[:, 0:1, 8:KX, :]
            )
            bmat = singles.tile([S, CPC, S], bf16)
            nc.sync.dma_start(out=bmat, in_=bmat_d[:, :, :])
            nc.scalar.dma_start(out=xt8[:, 1:4, :, :], in_=xt8_d[:, 1:4, :, :])
            nc.sync.dma_start(
                out=xt8[:, 4:CPC, :, :], in_=xt8_d[:, 4:CPC, :, :]
            )

            res = singles.tile([S, 2, CPC], f32, name="res")
            fst = res[:, 0, :]
            gst = res[:, 1, :]

            scrt = [
                singles.tile([S, S], f32, name=f"m{i}") for i in range(2)
            ]
            pss = [
                psp.tile([S, S], f32, padded_shape=[S, 512], name=f"ps{s}")
                for s in range(CPC)
            ]

            def gram_chain(s):
                for c in range(0, KX, 2):
                    nc.tensor.matmul(
                        pss[s],
                        xt8[:, s, c : c + 2, :],
                        xt8[:, s, c : c + 2, :],
                        start=False,
                        stop=(c == KX - 2),
                        perf_mode=DR,
                    )

            def reductions(s):
                nc.vector.tensor_reduce(
                    fst[:, s : s + 1], pss[s], axis=X, op=mybir.AluOpType.min
                )
                m = scrt[s % 2]
                nc.vector.tensor_add(m, pss[s], bmat[:, s, :])
                nc.vector.reduce_max(gst[:, s : s + 1], m, axis=X)

            def faug_mm(s):
                nc.tensor.matmul(
                    pss[s], ones2, faug[:, s, :], start=True, stop=False
                )

            # class 0 starts on slot0 alone; remaining aug matmuls slot in
            # behind it, buying the slot 1-3 DMAs time to land
            faug_mm(0)
            gram_chain(0)
            for s in range(1, CPC):
                faug_mm(s)
            reductions(0)
            for s in range(1, CPC):
                gram_chain(s)
                reductions(s)

            nc.scalar.dma_start(out=res_d[:, :, :], in_=res)

    nc.compile()
    return nc


def _prep_inputs(x, t):
    x = np.asarray(x, np.float32)
    t = np.asarray(t).astype(np.int64)
    sq = np.sum(x.astype(np.float64) ** 2, axis=1)

    order = np.argsort(t, kind="stable")
    sizes = np.bincount(t, minlength=NCLS)
    assert sizes.max() <= S, f"class size {sizes.max()} exceeds padding {S}"
    offs = np.zeros(NCLS + 1, np.int64)
    offs[1:] = np.cumsum(sizes)

    x8 = x.astype(F8)
    sqhalf = sq / 2.0
    hi = sqhalf.astype(BF)
    lo = (sqhalf - hi.astype(np.float64)).astype(BF)

    in_maps = []
    meta = []
    for core in range(NCORES):
        xt8_np = np.zeros((P, CPC, KX, S), F8)
        ofa_np = np.zeros((2, 1 + CPC, S), BF)
        ofa_np[:, 0, :] = BF(1.0)  # ones lhsT
        bmat_np = np.zeros((S, CPC, S), np.float32)
        cmeta = []
        for s in range(CPC):
            c = core * CPC + s
            idx = order[offs[c] : offs[c + 1]]
            n = len(idx)
            cmeta.append(idx)
            if n > 0:
                # [n, D] -> [D, n] -> [KX, P, n] -> [P, KX, n]
                blk = np.ascontiguousarray(x8[idx].T).reshape(KX, P, n)
                xt8_np[:, s, :, :n] = blk.transpose(1, 0, 2)
                ofa_np[0, 1 + s, :n] = -hi[idx]
                ofa_np[1, 1 + s, :n] = -lo[idx]
            ofa_np[0, 1 + s, n:] = BF(HUGE)
            bmat_np[np.arange(S), s, np.arange(S)] = -HUGE
            bmat_np[:n, s, n:] = -2 * HUGE
            bmat_np[n:, s, n:] += -2 * HUGE
        in_maps.append(
            {
                "xt8": xt8_np,
                "ofa": ofa_np,
                "bmat": bmat_np.astype(BF),
            }
        )
        meta.append(cmeta)
    return in_maps, meta, sq


def _assemble(results, meta, sq):
    far2 = np.empty(N, np.float64)
    near2 = np.empty(N, np.float64)
    for core in range(NCORES):
        r = np.asarray(results[core]["res"], np.float64)  # [S, 2, CPC]
        for s in range(CPC):
            idx = meta[core][s]
            n = len(idx)
            if n == 0:
                continue
            far2[idx] = sq[idx] - 2.0 * r[:n, 0, s]
            near2[idx] = sq[idx] - 2.0 * r[:n, 1, s]
    far = np.sqrt(np.maximum(far2, 1e-12))
    near = np.sqrt(np.maximum(near2, 1e-12))
    loss = np.float32(np.mean(np.maximum(far - near, 0.0)))
    return np.asarray(loss, np.float32)


def run_kernel(inputs, targets, trace=False):
    """Returns (loss, BassKernelResults)."""
    from concourse.bass_utils import run_bass_kernel_spmd

    global _compiled
    if _compiled is None:
        _compiled = _build_nc()
    nc = _compiled
    in_maps, meta, sq = _prep_inputs(inputs, targets)
    br = run_bass_kernel_spmd(
        nc, in_maps, core_ids=list(range(NCORES)), trace=trace
    )
    return _assemble(br.results, meta, sq), br


def kernel(inputs, targets):
    loss, _ = run_kernel(inputs, targets)
    return loss


# revision 34
# speedup vs baseline: 1.0754x; 1.0754x over previous
"""Trainium2 Bass kernel for a pairwise-distance cluster margin loss.

Key observation: the loss only ever reads SAME-CLASS distances (the mask
selects targets_i == targets_j for both the farthest-positive and the
second-nearest-positive), so the full 4096x4096 distance matrix is
unnecessary. After grouping points by class on the host, only the 64
class-diagonal Gram blocks (~82x82 max, padded to 96) are needed:
~64x less matmul work than the full GEMM.

Per class c (padded to S=96 points, zero-padded cols/rows):
    G    = x_c @ x_c.T                       (fp8 DoubleRow chain, PSUM f32)
    A    = G - sq_v/2 (bf16 hi/lo aug rows; pad cols get +2^20)
    fmin = rowmin(A)      -> far2_u  = sq_u - 2*fmin   (pad cols excluded
                             by +2^20; diag is +sq_u/2, never the min)
    M    = A + bmat        (bmat: diag -2^20, pad cols -2^21, via eye matmul)
    gmax = rowmax(M)      -> near2_u = sq_u - 2*gmax   (diag+pad pushed out)
Host: far=sqrt(max(far2,1e-12)), near=sqrt(max(near2,1e-12)),
loss = mean(relu(far-near)).  8 classes per core, 8 cores.
"""

import numpy as np
import ml_dtypes

BF = ml_dtypes.bfloat16
F8 = ml_dtypes.float8_e4m3

N = 4096
D = 2048
P = 128
NCORES = 8
NCLS = 64
S = 96  # padded class size (max observed 82)
CPC = NCLS // NCORES  # 8 classes per core
KX = D // P  # 16 k-chunks of 128
HUGE = float(2.0**20)

_compiled = None


def _build_nc():
    import concourse.mybir as mybir
    import concourse.tile as tile
    from concourse import bacc

    nc = bacc.Bacc("TRN2", target_bir_lowering=False)
    f32 = mybir.dt.float32
    bf16 = mybir.dt.bfloat16
    fp8 = mybir.dt.float8e4
    DR = mybir.MatmulPerfMode.DoubleRow
    X = mybir.AxisListType.X

    xt8_d = nc.dram_tensor("xt8", [P, CPC, KX, S], fp8, kind="ExternalInput")
    # row 0: ones lhsT rows; rows 1..CPC: per-class aug rhs rows
    ofa_d = nc.dram_tensor("ofa", [2, 1 + CPC, S], bf16, kind="ExternalInput")
    bmat_d = nc.dram_tensor("bmat", [S, CPC, S], bf16, kind="ExternalInput")
    res_d = nc.dram_tensor("res", [S, 2, CPC], f32, kind="ExternalOutput")

    with tile.TileContext(nc) as tc:
        with (
            tc.tile_pool(name="singles", bufs=1) as singles,
            tc.tile_pool(name="scr", bufs=2) as scr,
            tc.tile_pool(name="ps", bufs=1, space="PSUM") as psp,
        ):
            # two HWDGE queues, ordered by compute need-time: aug rows and
            # slot0 first, then slots interleaved, bmat before first tadd
            ofa = singles.tile([2, 1 + CPC, S], bf16)
            nc.sync.dma_start(out=ofa, in_=ofa_d[:, :, :])
            ones2 = ofa[:, 0, :]
            faug = ofa[:, 1:, :]
            xt8 = singles.tile([P, CPC, KX, S], fp8)
            nc.scalar.dma_start(
                out=xt8[:, 0:1, 0:8, :], in_=xt8_d[:, 0:1, 0:8, :]
            )
            nc.scalar.dma_start(
                out=xt8[:, 0:1, 8:KX, :], in_=xt8_d[:, 0:1, 8:KX, :]
            )
            def slot_dma(eng, s):
                eng.dma_start(
                    out=xt8[:, s : s + 1, :, :], in_=xt8_d[:, s : s + 1, :, :]
                )

            bmat = singles.tile([S, CPC, S], bf16)
            slot_dma(nc.sync, 1)
            slot_dma(nc.scalar, 2)
            nc.sync.dma_start(out=bmat, in_=bmat_d[:, :, :])
            slot_dma(nc.scalar, 5)
            slot_dma(nc.sync, 3)
            slot_dma(nc.scalar, 7)
            slot_dma(nc.sync, 4)
            slot_dma(nc.sync, 6)

            res = singles.tile([S, 2, CPC], f32, name="res")
            fst = res[:, 0, :]
            gst = res[:, 1, :]

            pss = [
                psp.tile([S, S], f32, padded_shape=[S, 512], name=f"ps{s}")
                for s in range(CPC)
            ]

            # phase A: per-class aug rows (one bf16 ldweights mode-switch)
            for s in range(CPC):
                nc.tensor.matmul(
                    pss[s], ones2, faug[:, s, :], start=True, stop=False
                )
            # phase B: fp8 DoubleRow Gram chains, then per-class reductions
            for s in range(CPC):
                ps = pss[s]
                for c in range(0, KX, 2):
                    nc.tensor.matmul(
                        ps,
                        xt8[:, s, c : c + 2, :],
                        xt8[:, s, c : c + 2, :],
                        start=False,
                        stop=(c == KX - 2),
                        perf_mode=DR,
                    )
                nc.vector.tensor_reduce(
                    fst[:, s : s + 1], ps, axis=X, op=mybir.AluOpType.min
                )
                m = scr.tile([S, S], f32)
                nc.vector.tensor_add(m, ps, bmat[:, s, :])
                nc.vector.reduce_max(gst[:, s : s + 1], m, axis=X)

            nc.sync.dma_start(out=res_d[:, 0:1, :], in_=res[:, 0:1, :])
            nc.scalar.dma_start(out=res_d[:, 1:2, :], in_=res[:, 1:2, :])

    nc.compile()
    return nc


def _prep_inputs(x, t):
    x = np.asarray(x, np.float32)
    t = np.asarray(t).astype(np.int64)
    sq = np.sum(x.astype(np.float64) ** 2, axis=1)

    order = np.argsort(t, kind="stable")
    sizes = np.bincount(t, minlength=NCLS)
    assert sizes.max() <= S, f"class size {sizes.max()} exceeds padding {S}"
    offs = np.zeros(NCLS + 1, np.int64)
    offs[1:] = np.cumsum(sizes)

    x8 = x.astype(F8)
    sqhalf = sq / 2.0
    hi = sqhalf.astype(BF)
    lo = (sqhalf - hi.astype(np.float64)).astype(BF)

    in_maps = []
    meta = []
    for core in range(NCORES):
        xt8_np = np.zeros((P, CPC, KX, S), F8)
        ofa_np = np.zeros((2, 1 + CPC, S), BF)
        ofa_np[:, 0, :] = BF(1.0)  # ones lhsT
        bmat_np = np.zeros((S, CPC, S), np.float32)
        cmeta = []
        for s in range(CPC):
            c = core * CPC + s
            idx = order[offs[c] : offs[c + 1]]
            n = len(idx)
            cmeta.append(idx)
            if n > 0:
                # [n, D] -> [D, n] -> [KX, P, n] -> [P, KX, n]
                blk = np.ascontiguousarray(x8[idx].T).reshape(KX, P, n)
                xt8_np[:, s, :, :n] = blk.transpose(1, 0, 2)
                ofa_np[0, 1 + s, :n] = -hi[idx]
                ofa_np[1, 1 + s, :n] = -lo[idx]
            ofa_np[0, 1 + s, n:] = BF(HUGE)
            bmat_np[np.arange(S), s, np.arange(S)] = -HUGE
            bmat_np[:n, s, n:] = -2 * HUGE
            bmat_np[n:, s, n:] += -2 * HUGE
        in_maps.append(
            {
                "xt8": xt8_np,
                "ofa": ofa_np,
                "bmat": bmat_np.astype(BF),
            }
        )
        meta.append(cmeta)
    return in_maps, meta, sq


def _assemble(results, meta, sq):
    far2 = np.empty(N, np.float64)
    near2 = np.empty(N, np.float64)
    for core in range(NCORES):
        r = np.asarray(results[core]["res"], np.float64)  # [S, 2, CPC]
        for s in range(CPC):
            idx = meta[core][s]
            n = len(idx)
            if n == 0:
                continue
            far2[idx] = sq[idx] - 2.0 * r[:n, 0, s]
            near2[idx] = sq[idx] - 2.0 * r[:n, 1, s]
    far = np.sqrt(np.maximum(far2, 1e-12))
    near = np.sqrt(np.maximum(near2, 1e-12))
    loss = np.float32(np.mean(np.maximum(far - near, 0.0)))
    return np.asarray(loss, np.float32)


def run_kernel(inputs, targets, trace=False):
    """Returns (loss, BassKernelResults)."""
    from concourse.bass_utils import run_bass_kernel_spmd

    global _compiled
    if _compiled is None:
        _compiled = _build_nc()
    nc = _compiled
    in_maps, meta, sq = _prep_inputs(inputs, targets)
    br = run_bass_kernel_spmd(
        nc, in_maps, core_ids=list(range(NCORES)), trace=trace
    )
    return _assemble(br.results, meta, sq), br


def kernel(inputs, targets):
    loss, _ = run_kernel(inputs, targets)
    return loss


# revision 36
# speedup vs baseline: 1.0919x; 1.0154x over previous
"""Trainium2 Bass kernel for a pairwise-distance cluster margin loss.

Key observation: the loss only ever reads SAME-CLASS distances (the mask
selects targets_i == targets_j for both the farthest-positive and the
second-nearest-positive), so the full 4096x4096 distance matrix is
unnecessary. After grouping points by class on the host, only the 64
class-diagonal Gram blocks (~82x82 max, padded to 96) are needed:
~64x less matmul work than the full GEMM.

Per class c (padded to S=96 points, zero-padded cols/rows):
    G    = x_c @ x_c.T                       (fp8 DoubleRow chain, PSUM f32)
    A    = G - sq_v/2 (bf16 hi/lo aug rows; pad cols get +2^20)
    fmin = rowmin(A)      -> far2_u  = sq_u - 2*fmin   (pad cols excluded
                             by +2^20; diag is +sq_u/2, never the min)
    M    = A + bmat        (bmat: diag -2^20, pad cols -2^21, via eye matmul)
    gmax = rowmax(M)      -> near2_u = sq_u - 2*gmax   (diag+pad pushed out)
Host: far=sqrt(max(far2,1e-12)), near=sqrt(max(near2,1e-12)),
loss = mean(relu(far-near)).  8 classes per core, 8 cores.
"""

import numpy as np
import ml_dtypes

BF = ml_dtypes.bfloat16
F8 = ml_dtypes.float8_e4m3

N = 4096
D = 2048
P = 128
NCORES = 8
NCLS = 64
S = 96  # padded class size (max observed 82)
CPC = NCLS // NCORES  # 8 classes per core
KX = D // P  # 16 k-chunks of 128
HUGE = float(2.0**20)

_compiled = None


def _build_nc():
    import concourse.mybir as mybir
    import concourse.tile as tile
    from concourse import bacc

    nc = bacc.Bacc("TRN2", target_bir_lowering=False)
    f32 = mybir.dt.float32
    bf16 = mybir.dt.bfloat16
    fp8 = mybir.dt.float8e4
    DR = mybir.MatmulPerfMode.DoubleRow
    X = mybir.AxisListType.X

    xt8_d = nc.dram_tensor("xt8", [P, CPC, KX, S], fp8, kind="ExternalInput")
    # row 0: ones lhsT rows; rows 1..CPC: per-class aug rhs rows
    ofa_d = nc.dram_tensor("ofa", [2, 1 + CPC, S], bf16, kind="ExternalInput")
    bmat_d = nc.dram_tensor("bmat", [S, CPC, S], bf16, kind="ExternalInput")
    res_d = nc.dram_tensor("res", [S, 2, CPC], f32, kind="ExternalOutput")

    with tile.TileContext(nc) as tc:
        with (
            tc.tile_pool(name="singles", bufs=1) as singles,
            tc.tile_pool(name="scr", bufs=2) as scr,
            tc.tile_pool(name="ps", bufs=1, space="PSUM") as psp,
        ):
            # two HWDGE queues, ordered by compute need-time: aug rows and
            # slot0 first, then slots interleaved, bmat before first tadd
            ofa = singles.tile([2, 1 + CPC, S], bf16)
            nc.sync.dma_start(out=ofa, in_=ofa_d[:, :, :])
            ones2 = ofa[:, 0, :]
            faug = ofa[:, 1:, :]
            xt8 = singles.tile([P, CPC, KX, S], fp8)
            nc.scalar.dma_start(
                out=xt8[:, 0:1, 0:8, :], in_=xt8_d[:, 0:1, 0:8, :]
            )
            nc.scalar.dma_start(
                out=xt8[:, 0:1, 8:KX, :], in_=xt8_d[:, 0:1, 8:KX, :]
            )
            def slot_dma(eng, s):
                eng.dma_start(
                    out=xt8[:, s : s + 1, :, :], in_=xt8_d[:, s : s + 1, :, :]
                )

            bmat = singles.tile([S, CPC, S], bf16)
            slot_dma(nc.sync, 1)
            slot_dma(nc.scalar, 2)
            nc.sync.dma_start(out=bmat, in_=bmat_d[:, :, :])
            slot_dma(nc.scalar, 5)
            slot_dma(nc.sync, 3)
            slot_dma(nc.scalar, 7)
            slot_dma(nc.sync, 4)
            slot_dma(nc.sync, 6)

            res = singles.tile([S, 2, CPC], f32, name="res")
            fst = res[:, 0, :]
            gst = res[:, 1, :]

            # two classes per PSUM bank: halves aug matmuls and DVE ops
            NB = CPC // 2
            pss = [
                psp.tile([S, 2, S], f32, padded_shape=[S, 2, 256], name=f"ps{b}")
                for b in range(NB)
            ]

            # phase A: aug rows for class pairs (one bf16 mode-switch)
            for b in range(NB):
                nc.tensor.matmul(
                    pss[b],
                    ones2,
                    faug[:, 2 * b : 2 * b + 2, :],
                    start=True,
                    stop=False,
                )
            # phase B: fp8 DoubleRow Gram chains, then per-pair reductions
            for s in range(CPC):
                b, k = divmod(s, 2)
                ps = pss[b]
                for c in range(0, KX, 2):
                    nc.tensor.matmul(
                        ps[:, k, :],
                        xt8[:, s, c : c + 2, :],
                        xt8[:, s, c : c + 2, :],
                        start=False,
                        stop=(c == KX - 2 and k == 1),
                        perf_mode=DR,
                        skip_group_check=True,
                    )
                if k == 1:
                    nc.vector.tensor_reduce(
                        fst[:, 2 * b : 2 * b + 2],
                        ps,
                        axis=X,
                        op=mybir.AluOpType.min,
                    )
                    m = scr.tile([S, 2, S], f32)
                    nc.vector.tensor_add(m, ps, bmat[:, 2 * b : 2 * b + 2, :])
                    nc.vector.reduce_max(gst[:, 2 * b : 2 * b + 2], m, axis=X)

            nc.sync.dma_start(out=res_d[:, 0:1, :], in_=res[:, 0:1, :])
            nc.scalar.dma_start(out=res_d[:, 1:2, :], in_=res[:, 1:2, :])

    nc.compile()
    return nc


def _prep_inputs(x, t):
    x = np.asarray(x, np.float32)
    t = np.asarray(t).astype(np.int64)
    sq = np.sum(x.astype(np.float64) ** 2, axis=1)

    order = np.argsort(t, kind="stable")
    sizes = np.bincount(t, minlength=NCLS)
    assert sizes.max() <= S, f"class size {sizes.max()} exceeds padding {S}"
    offs = np.zeros(NCLS + 1, np.int64)
    offs[1:] = np.cumsum(sizes)

    x8 = x.astype(F8)
    sqhalf = sq / 2.0
    hi = sqhalf.astype(BF)
    lo = (sqhalf - hi.astype(np.float64)).astype(BF)

    in_maps = []
    meta = []
    for core in range(NCORES):
        xt8_np = np.zeros((P, CPC, KX, S), F8)
        ofa_np = np.zeros((2, 1 + CPC, S), BF)
        ofa_np[:, 0, :] = BF(1.0)  # ones lhsT
        bmat_np = np.zeros((S, CPC, S), np.float32)
        cmeta = []
        for s in range(CPC):
            c = core * CPC + s
            idx = order[offs[c] : offs[c + 1]]
            n = len(idx)
            cmeta.append(idx)
            if n > 0:
                # [n, D] -> [D, n] -> [KX, P, n] -> [P, KX, n]
                blk = np.ascontiguousarray(x8[idx].T).reshape(KX, P, n)
                xt8_np[:, s, :, :n] = blk.transpose(1, 0, 2)
                ofa_np[0, 1 + s, :n] = -hi[idx]
                ofa_np[1, 1 + s, :n] = -lo[idx]
            ofa_np[0, 1 + s, n:] = BF(HUGE)
            bmat_np[np.arange(S), s, np.arange(S)] = -HUGE
            bmat_np[:n, s, n:] = -2 * HUGE
            bmat_np[n:, s, n:] += -2 * HUGE
        in_maps.append(
            {
                "xt8": xt8_np,
                "ofa": ofa_np,
                "bmat": bmat_np.astype(BF),
            }
        )
        meta.append(cmeta)
    return in_maps, meta, sq


def _assemble(results, meta, sq):
    far2 = np.empty(N, np.float64)
    near2 = np.empty(N, np.float64)
    for core in range(NCORES):
        r = np.asarray(results[core]["res"], np.float64)  # [S, 2, CPC]
        for s in range(CPC):
            idx = meta[core][s]
            n = len(idx)
            if n == 0:
                continue
            far2[idx] = sq[idx] - 2.0 * r[:n, 0, s]
            near2[idx] = sq[idx] - 2.0 * r[:n, 1, s]
    far = np.sqrt(np.maximum(far2, 1e-12))
    near = np.sqrt(np.maximum(near2, 1e-12))
    loss = np.float32(np.mean(np.maximum(far - near, 0.0)))
    return np.asarray(loss, np.float32)


def run_kernel(inputs, targets, trace=False):
    """Returns (loss, BassKernelResults)."""
    from concourse.bass_utils import run_bass_kernel_spmd

    global _compiled
    if _compiled is None:
        _compiled = _build_nc()
    nc = _compiled
    in_maps, meta, sq = _prep_inputs(inputs, targets)
    br = run_bass_kernel_spmd(
        nc, in_maps, core_ids=list(range(NCORES)), trace=trace
    )
    return _assemble(br.results, meta, sq), br


def kernel(inputs, targets):
    loss, _ = run_kernel(inputs, targets)
    return loss


# revision 38
# speedup vs baseline: 1.0945x; 1.0024x over previous
"""Trainium2 Bass kernel for a pairwise-distance cluster margin loss.

Key observation: the loss only ever reads SAME-CLASS distances (the mask
selects targets_i == targets_j for both the farthest-positive and the
second-nearest-positive), so the full 4096x4096 distance matrix is
unnecessary. After grouping points by class on the host, only the 64
class-diagonal Gram blocks (~82x82 max, padded to 96) are needed:
~64x less matmul work than the full GEMM.

Per class c (padded to S=96 points, zero-padded cols/rows):
    G    = x_c @ x_c.T                       (fp8 DoubleRow chain, PSUM f32)
    A    = G - sq_v/2 (bf16 hi/lo aug rows; pad cols get +2^20)
    fmin = rowmin(A)      -> far2_u  = sq_u - 2*fmin   (pad cols excluded
                             by +2^20; diag is +sq_u/2, never the min)
    M    = A + bmat        (bmat: diag -2^20, pad cols -2^21, via eye matmul)
    gmax = rowmax(M)      -> near2_u = sq_u - 2*gmax   (diag+pad pushed out)
Host: far=sqrt(max(far2,1e-12)), near=sqrt(max(near2,1e-12)),
loss = mean(relu(far-near)).  8 classes per core, 8 cores.
"""

import numpy as np
import ml_dtypes

BF = ml_dtypes.bfloat16
F8 = ml_dtypes.float8_e4m3

N = 4096
D = 2048
P = 128
NCORES = 8
NCLS = 64
S = 96  # padded class size (max observed 82; ldweights needs S%32==0)
CPC = NCLS // NCORES  # 8 classes per core
KX = D // P  # 16 k-chunks of 128
HUGE = float(2.0**20)

_compiled = None


def _build_nc():
    import concourse.mybir as mybir
    import concourse.tile as tile
    from concourse import bacc

    nc = bacc.Bacc("TRN2", target_bir_lowering=False)
    f32 = mybir.dt.float32
    bf16 = mybir.dt.bfloat16
    fp8 = mybir.dt.float8e4
    DR = mybir.MatmulPerfMode.DoubleRow
    X = mybir.AxisListType.X

    xt8_d = nc.dram_tensor("xt8", [P, CPC, KX, S], fp8, kind="ExternalInput")
    # row 0: ones lhsT rows; rows 1..CPC: per-class aug rhs rows
    ofa_d = nc.dram_tensor("ofa", [2, 1 + CPC, S], bf16, kind="ExternalInput")
    bmat_d = nc.dram_tensor("bmat", [S, CPC, S], bf16, kind="ExternalInput")
    res_d = nc.dram_tensor("res", [S, 2, CPC], f32, kind="ExternalOutput")

    with tile.TileContext(nc) as tc:
        with (
            tc.tile_pool(name="singles", bufs=1) as singles,
            tc.tile_pool(name="scr", bufs=2) as scr,
            tc.tile_pool(name="ps", bufs=1, space="PSUM") as psp,
        ):
            # two HWDGE queues, ordered by compute need-time: aug rows and
            # slot0 first, then slots interleaved, bmat before first tadd
            ofa = singles.tile([2, 1 + CPC, S], bf16)
            nc.sync.dma_start(out=ofa, in_=ofa_d[:, :, :])
            ones2 = ofa[:, 0, :]
            faug = ofa[:, 1:, :]
            xt8 = singles.tile([P, CPC, KX, S], fp8)
            nc.scalar.dma_start(
                out=xt8[:, 0:1, 0:8, :], in_=xt8_d[:, 0:1, 0:8, :]
            )
            nc.scalar.dma_start(
                out=xt8[:, 0:1, 8:KX, :], in_=xt8_d[:, 0:1, 8:KX, :]
            )
            def slot_dma(eng, s):
                eng.dma_start(
                    out=xt8[:, s : s + 1, :, :], in_=xt8_d[:, s : s + 1, :, :]
                )

            bmat = singles.tile([S, CPC, S], bf16)
            slot_dma(nc.sync, 1)
            slot_dma(nc.scalar, 2)
            nc.sync.dma_start(out=bmat, in_=bmat_d[:, :, :])
            slot_dma(nc.scalar, 5)
            slot_dma(nc.sync, 3)
            slot_dma(nc.scalar, 7)
            slot_dma(nc.sync, 4)
            slot_dma(nc.sync, 6)

            res = singles.tile([S, 2, CPC], f32, name="res")
            fst = res[:, 0, :]
            gst = res[:, 1, :]

            # two classes per PSUM bank: halves aug matmuls and DVE ops
            NB = CPC // 2
            pss = [
                psp.tile([S, 2, S], f32, padded_shape=[S, 2, 256], name=f"ps{b}")
                for b in range(NB)
            ]

            # phase A: aug rows for class pairs (one bf16 mode-switch)
            for b in range(NB):
                nc.tensor.matmul(
                    pss[b],
                    ones2,
                    faug[:, 2 * b : 2 * b + 2, :],
                    start=True,
                    stop=False,
                )
            # phase B: fp8 DoubleRow Gram chains, then per-pair reductions
            for s in range(CPC):
                b, k = divmod(s, 2)
                ps = pss[b]
                for c in range(0, KX, 2):
                    nc.tensor.matmul(
                        ps[:, k, :],
                        xt8[:, s, c : c + 2, :],
                        xt8[:, s, c : c + 2, :],
                        start=False,
                        stop=(c == KX - 2 and k == 1),
                        perf_mode=DR,
                        skip_group_check=True,
                    )
                if k == 1:
                    nc.vector.tensor_reduce(
                        fst[:, 2 * b : 2 * b + 2],
                        ps,
                        axis=X,
                        op=mybir.AluOpType.min,
                    )
                    m = scr.tile([S, 2, S], f32)
                    nc.vector.tensor_add(m, ps, bmat[:, 2 * b : 2 * b + 2, :])
                    nc.vector.reduce_max(gst[:, 2 * b : 2 * b + 2], m, axis=X)

            nc.sync.dma_start(out=res_d[:, 0:1, :], in_=res[:, 0:1, :])
            nc.scalar.dma_start(out=res_d[:, 1:2, :], in_=res[:, 1:2, :])

    nc.compile()
    return nc


def _prep_inputs(x, t):
    x = np.asarray(x, np.float32)
    t = np.asarray(t).astype(np.int64)
    sq = np.sum(x.astype(np.float64) ** 2, axis=1)

    order = np.argsort(t, kind="stable")
    sizes = np.bincount(t, minlength=NCLS)
    assert sizes.max() <= S, f"class size {sizes.max()} exceeds padding {S}"
    offs = np.zeros(NCLS + 1, np.int64)
    offs[1:] = np.cumsum(sizes)

    x8 = x.astype(F8)
    sqhalf = sq / 2.0
    hi = sqhalf.astype(BF)
    lo = (sqhalf - hi.astype(np.float64)).astype(BF)

    in_maps = []
    meta = []
    for core in range(NCORES):
        xt8_np = np.zeros((P, CPC, KX, S), F8)
        ofa_np = np.zeros((2, 1 + CPC, S), BF)
        ofa_np[:, 0, :] = BF(1.0)  # ones lhsT
        bmat_np = np.zeros((S, CPC, S), np.float32)
        cmeta = []
        for s in range(CPC):
            c = core * CPC + s
            idx = order[offs[c] : offs[c + 1]]
            n = len(idx)
            cmeta.append(idx)
            if n > 0:
                # [n, D] -> [D, n] -> [KX, P, n] -> [P, KX, n]
                blk = np.ascontiguousarray(x8[idx].T).reshape(KX, P, n)
                xt8_np[:, s, :, :n] = blk.transpose(1, 0, 2)
                ofa_np[0, 1 + s, :n] = -hi[idx]
                ofa_np[1, 1 + s, :n] = -lo[idx]
            ofa_np[0, 1 + s, n:] = BF(HUGE)
            bmat_np[np.arange(S), s, np.arange(S)] = -HUGE
            bmat_np[:n, s, n:] = -2 * HUGE
            bmat_np[n:, s, n:] += -2 * HUGE
        in_maps.append(
            {
                "xt8": xt8_np,
                "ofa": ofa_np,
                "bmat": bmat_np.astype(BF),
            }
        )
        meta.append(cmeta)
    return in_maps, meta, sq


def _assemble(results, meta, sq):
    far2 = np.empty(N, np.float64)
    near2 = np.empty(N, np.float64)
    for core in range(NCORES):
        r = np.asarray(results[core]["res"], np.float64)  # [S, 2, CPC]
        for s in range(CPC):
            idx = meta[core][s]
            n = len(idx)
            if n == 0:
                continue
            far2[idx] = sq[idx] - 2.0 * r[:n, 0, s]
            near2[idx] = sq[idx] - 2.0 * r[:n, 1, s]
    far = np.sqrt(np.maximum(far2, 1e-12))
    near = np.sqrt(np.maximum(near2, 1e-12))
    loss = np.float32(np.mean(np.maximum(far - near, 0.0)))
    return np.asarray(loss, np.float32)


def run_kernel(inputs, targets, trace=False):
    """Returns (loss, BassKernelResults)."""
    from concourse.bass_utils import run_bass_kernel_spmd

    global _compiled
    if _compiled is None:
        _compiled = _build_nc()
    nc = _compiled
    in_maps, meta, sq = _prep_inputs(inputs, targets)
    br = run_bass_kernel_spmd(
        nc, in_maps, core_ids=list(range(NCORES)), trace=trace
    )
    return _assemble(br.results, meta, sq), br


def kernel(inputs, targets):
    loss, _ = run_kernel(inputs, targets)
    return loss
